# revision 1
# baseline (speedup 1.0000x reference)
"""Multi-head attention (b=4, n=2048, dim=512, h=8, dh=64) on 8 trn2 cores.

Sharding: core c handles batch b=c//2 and query rows
[half*1024, (half+1)*1024) with half=c%2. K/V (from x_prev) are computed
redundantly on both of a batch's cores (cheap vs. attention). No collectives.

Per-core kernel (bf16 operands, fp32 PSUM accumulation):
  QT[inner, nq]  = w_q-tiles  (lhsT) @ x^T          (q in transposed layout)
  KT[inner, nk]  = w_kv-tiles (lhsT) @ x_prev^T
  V [nk, inner]  = x_prev^T-tiles (lhsT) @ w_kv[:, v]  (natural layout,
                                                        + ones column/head)
  ST[j, i]       = KT_h-tile (lhsT, K=dh=64) @ QT_h  (scores transposed;
                   two heads row-tiled in the PE at partitions 0/64)
  PT             = exp(ST * scale)    (no max subtraction: |s*scale| < ~8)
  PV[dh+1, i]    = sum_j V_h|ones (lhsT) @ PT_h      (row dh = sum_j P = l)
  l -> (dma to partitions 0/1) -> r = 1/l -> rb_h = ones-matmul broadcast
  AOT[0:64, h]   = PV[0:dh] * rb_h                   (per-head, partitions 0-63)
  out[i, d]      = sum_h AOT_h-tiles (lhsT, K=64) @ w_out_h + ones @ b_out
"""

import numpy as np
import ml_dtypes

B, N, DIM = 4, 2048, 512
H, DH, INNER = 8, 64, 512
NCORES = 8

_BUILT = None


def build_module(dim=DIM, h=H, nq=N // 2, nk=N, compile_module=True, reps=1,
                 stub=frozenset()):
    """Build the per-core attention module. dim % 128 == 0, h % 2 == 0,
    nq % 512 == 0, nk % 128 == 0. reps>1 repeats the whole compute body
    (timing calibration only)."""
    import concourse.mybir as mybir
    import concourse.tile as tile
    from concourse import bacc

    CDT = mybir.dt.bfloat16
    FDT = mybir.dt.float32
    Exp = mybir.ActivationFunctionType.Exp

    inner = h * DH
    nkt = dim // 128          # contraction tiles for projections
    npr = h // 2              # head pairs (= inner // 128 slices of KT/QT)
    nj = nk // 128            # key tiles
    nqc = nq // 512           # query chunks
    VW = DH + 1               # 65: per-head v columns + ones column
    scale = DH ** -0.5

    nc = bacc.Bacc("TRN2", target_bir_lowering=False, debug=False,
                   num_devices=NCORES)

    xt_d = nc.declare_dram_parameter("xt", [dim, nq], CDT, isOutput=False)
    xpt_d = nc.declare_dram_parameter("xpt", [dim, nk], CDT, isOutput=False)
    wq_d = nc.declare_dram_parameter("wq", [dim, inner], CDT, isOutput=False)
    wkv_d = nc.declare_dram_parameter("wkv", [dim, 2 * inner], CDT,
                                      isOutput=False)
    # w_out pre-arranged on host: wout2[d, h, :] = w_out[h*64+d, :]
    wout_d = nc.declare_dram_parameter("wout", [DH, h, dim], CDT,
                                       isOutput=False)
    bout_d = nc.declare_dram_parameter("bout", [1, dim], CDT, isOutput=False)
    out_d = nc.declare_dram_parameter("out", [nq, dim], FDT, isOutput=True)
    # DRAM bounce rows for the 1/l partition-broadcast (SBUF APs cannot
    # have a zero-step partition dim; DRAM APs can).
    rsc_d = nc.dram_tensor("rscratch", [h * nq // 512, 512], FDT)

    import contextlib
    with tile.TileContext(nc) as tc, contextlib.ExitStack() as stack:
        consts = stack.enter_context(tc.tile_pool(name="consts", bufs=1))
        acts = stack.enter_context(tc.tile_pool(name="acts", bufs=1))

        # ---- constants / weights ----------------------------------------
        wq_sb = consts.tile([128, nkt, inner], CDT)
        wkv_sb = consts.tile([128, nkt, 2 * inner], CDT)
        wout_sb = consts.tile([DH, h, dim], CDT)
        bout_sb = consts.tile([1, dim], CDT)
        ones_sb = consts.tile([1, 128], CDT)

        for k in range(nkt):
            nc.sync.dma_start(
                out=wq_sb[:, k, :],
                in_=wq_d.ap().rearrange("(t p) o -> p t o", p=128)[:, k, :])
            nc.sync.dma_start(
                out=wkv_sb[:, k, :],
                in_=wkv_d.ap().rearrange("(t p) o -> p t o", p=128)[:, k, :])
        nc.sync.dma_start(out=wout_sb[:, :, :], in_=wout_d.ap())
        nc.sync.dma_start(out=bout_sb[:, :], in_=bout_d.ap())
        nc.vector.memset(ones_sb[:, :], 1.0)

        # ---- activations -------------------------------------------------
        xt_sb = acts.tile([128, nkt, nq], CDT)
        xpt_sb = acts.tile([128, nkt, nk], CDT)
        for k in range(nkt):
            nc.sync.dma_start(
                out=xt_sb[:, k, :],
                in_=xt_d.ap().rearrange("(t p) n -> p t n", p=128)[:, k, :])
            nc.sync.dma_start(
                out=xpt_sb[:, k, :],
                in_=xpt_d.ap().rearrange("(t p) n -> p t n", p=128)[:, k, :])

        qt_sb = acts.tile([128, npr, nq], CDT)    # [inner-slice, nq]
        kt_sb = acts.tile([128, npr, nk], CDT)    # [inner-slice, nk]
        v_sb = acts.tile([128, nj, h * VW], CDT)  # [key-tile, h*(dh+1)]
        aot_sb = acts.tile([DH, h, nq], CDT)      # [dh, head, nq]

        for hh in range(h):  # ones columns of V
            nc.vector.memset(v_sb[:, :, hh * VW + DH:hh * VW + DH + 1], 1.0)

        for _rep in range(reps):
            # KT/QT slice 0 first so attention can start early; V interleaved.
            kw = min(512, nk)   # kt projection chunk width
            order = []
            for s in range(npr):
                for c in range(nk // kw):
                    order.append(("kt", s, c))
                for c in range(nqc):
                    order.append(("qt", s, c))
                if s == 0:
                    for j in range(nj):
                        order.append(("v", j, 0))
            proj_scope = tc.tile_pool(name="proj_ps", bufs=4, space="PSUM")
            proj_ps = proj_scope.__enter__()
            for kind, a, c in order:
                ps = proj_ps.tile([128, 512], FDT, tag="mm")
                if kind == "kt":
                    for k in range(nkt):
                        nc.tensor.matmul(
                            ps[:, 0:kw], lhsT=wkv_sb[:, k, a * 128:(a + 1) * 128],
                            rhs=xpt_sb[:, k, c * kw:(c + 1) * kw],
                            start=(k == 0), stop=(k == nkt - 1))
                    nc.vector.tensor_copy(
                        out=kt_sb[:, a, c * kw:(c + 1) * kw], in_=ps[:, 0:kw])
                elif kind == "qt":
                    for k in range(nkt):
                        nc.tensor.matmul(
                            ps[:, :], lhsT=wq_sb[:, k, a * 128:(a + 1) * 128],
                            rhs=xt_sb[:, k, c * 512:(c + 1) * 512],
                            start=(k == 0), stop=(k == nkt - 1))
                    nc.vector.tensor_copy(
                        out=qt_sb[:, a, c * 512:(c + 1) * 512], in_=ps[:, :])
                else:  # v: natural layout, lhsT = xpt token-tile
                    for k in range(nkt):
                        nc.tensor.matmul(
                            ps[:, 0:inner],
                            lhsT=xpt_sb[:, k, a * 128:(a + 1) * 128],
                            rhs=wkv_sb[:, k, inner:2 * inner],
                            start=(k == 0), stop=(k == nkt - 1))
                    nc.vector.tensor_copy(
                        out=v_sb[:, a, :].rearrange(
                            "p (g x) -> p g x", x=VW)[:, :, 0:DH],
                        in_=ps[:, 0:inner].rearrange("p (g x) -> p g x", x=DH))

            proj_scope.__exit__(None, None, None)

            # ---- attention ---------------------------------------------------
            attn_stack = contextlib.ExitStack()
            st_ps = attn_stack.enter_context(
                tc.tile_pool(name="st_ps", bufs=2, space="PSUM"))
            acc_ps = attn_stack.enter_context(
                tc.tile_pool(name="acc_ps", bufs=4, space="PSUM"))
            pt_pool = attn_stack.enter_context(tc.tile_pool(name="pt", bufs=2))
            lr_pool = attn_stack.enter_context(tc.tile_pool(name="lr", bufs=3))

            for c in range(nqc):          # query chunk of 512
                for p in range(npr):      # head pair (2p, 2p+1)
                    h0, h1 = 2 * p, 2 * p + 1
                    pt = pt_pool.tile([128, nj, 1024], CDT, tag="pt")
                    for j in range(nj):
                        st = st_ps.tile([128, 1024], FDT, tag="st")
                        nc.tensor.matmul(
                            st[:, 0:512],
                            lhsT=kt_sb[0:64, p, j * 128:(j + 1) * 128],
                            rhs=qt_sb[0:64, p, c * 512:(c + 1) * 512],
                            start=True, stop=True)
                        nc.tensor.matmul(
                            st[:, 512:1024],
                            lhsT=kt_sb[64:128, p, j * 128:(j + 1) * 128],
                            rhs=qt_sb[64:128, p, c * 512:(c + 1) * 512],
                            start=True, stop=True)
                        if "noexp" in stub:
                            nc.vector.tensor_copy(out=pt[:, j, :],
                                                  in_=st[:, :])
                        else:
                            nc.scalar.activation(out=pt[:, j, :], in_=st[:, :],
                                                 func=Exp, scale=scale)

                    pv0 = acc_ps.tile([128, 512], FDT, tag="acc")
                    pv1 = acc_ps.tile([128, 512], FDT, tag="acc")
                    for j in range(nj):
                        nc.tensor.matmul(
                            pv0[0:VW, :], lhsT=v_sb[:, j, h0 * VW:(h0 + 1) * VW],
                            rhs=pt[:, j, 0:512],
                            start=(j == 0), stop=(j == nj - 1))
                        nc.tensor.matmul(
                            pv1[0:VW, :], lhsT=v_sb[:, j, h1 * VW:(h1 + 1) * VW],
                            rhs=pt[:, j, 512:1024],
                            start=(j == 0), stop=(j == nj - 1))

                    if "nonorm" in stub:
                        nc.vector.tensor_copy(
                            out=aot_sb[:, h0, c * 512:(c + 1) * 512],
                            in_=pv0[0:DH, :])
                        nc.vector.tensor_copy(
                            out=aot_sb[:, h1, c * 512:(c + 1) * 512],
                            in_=pv1[0:DH, :])
                        continue
                    # softmax denominators: 1/l on lane DH, then a step-0
                    # partition DMA broadcasts it to 64 partitions in SBUF.
                    ra_sb = lr_pool.tile([DH + 1, 512], FDT, tag="ra")
                    rc_sb = lr_pool.tile([DH + 1, 512], FDT, tag="rc")
                    nc.vector.reciprocal(out=ra_sb[DH:DH + 1, :],
                                         in_=pv0[DH:DH + 1, :])
                    nc.vector.reciprocal(out=rc_sb[DH:DH + 1, :],
                                         in_=pv1[DH:DH + 1, :])
                    idx = (c * npr + p) * 2
                    nc.sync.dma_start(out=rsc_d.ap()[idx:idx + 1, :],
                                      in_=ra_sb[DH:DH + 1, :])
                    nc.sync.dma_start(out=rsc_d.ap()[idx + 1:idx + 2, :],
                                      in_=rc_sb[DH:DH + 1, :])
                    rb0 = lr_pool.tile([DH, 512], FDT, tag="rb0")
                    rb1 = lr_pool.tile([DH, 512], FDT, tag="rb1")
                    nc.gpsimd.dma_start(
                        out=rb0[:, :],
                        in_=rsc_d.ap()[idx:idx + 1, :].to_broadcast([DH, 512]))
                    nc.gpsimd.dma_start(
                        out=rb1[:, :],
                        in_=rsc_d.ap()[idx + 1:idx + 2, :].to_broadcast([DH, 512]))
                    nc.vector.tensor_mul(
                        aot_sb[:, h0, c * 512:(c + 1) * 512],
                        pv0[0:DH, :], rb0[:, :])
                    nc.vector.tensor_mul(
                        aot_sb[:, h1, c * 512:(c + 1) * 512],
                        pv1[0:DH, :], rb1[:, :])

                # ---- output projection for this chunk (4 row-tiles of 128) ---
                for t in range(4 * c, 4 * c + 4):
                    f = acc_ps.tile([128, 512], FDT, tag="acc")
                    for hh in range(h):
                        nc.tensor.matmul(
                            f[:, 0:dim],
                            lhsT=aot_sb[:, hh, t * 128:(t + 1) * 128],
                            rhs=wout_sb[:, hh, :],
                            start=(hh == 0), stop=False)
                    nc.tensor.matmul(f[:, 0:dim], lhsT=ones_sb[:, :],
                                     rhs=bout_sb[:, :], start=False, stop=True)
                    fo = lr_pool.tile([128, dim], FDT, tag="fo")
                    nc.vector.tensor_copy(out=fo[:, :], in_=f[:, 0:dim])
                    nc.sync.dma_start(
                        out=out_d.ap()[t * 128:(t + 1) * 128, :], in_=fo[:, :])
            attn_stack.close()

    if compile_module:
        nc.compile()
    return nc


def host_inputs(x, x_prev, w_q, w_kv, w_out, b_out, ncores=NCORES):
    """Shard + lay out the full inputs into per-core input maps."""
    bf16 = ml_dtypes.bfloat16
    b, n, dim = x.shape
    inner = w_q.shape[1]
    h = inner // DH
    nq = (b * n) // ncores
    halves = ncores // b
    wq = np.ascontiguousarray(w_q).astype(bf16)
    wkv = np.ascontiguousarray(w_kv).astype(bf16)
    wout = np.ascontiguousarray(
        w_out.reshape(h, DH, dim).transpose(1, 0, 2)).astype(bf16)
    bout = np.ascontiguousarray(b_out).reshape(1, dim).astype(bf16)
    in_maps = []
    for c in range(ncores):
        bb, half = c // halves, c % halves
        xt = np.ascontiguousarray(
            x[bb, half * nq:(half + 1) * nq, :].T).astype(bf16)
        xpt = np.ascontiguousarray(x_prev[bb].T).astype(bf16)
        in_maps.append(dict(xt=xt, xpt=xpt, wq=wq, wkv=wkv, wout=wout,
                            bout=bout))
    return in_maps


def _get_module():
    global _BUILT
    if _BUILT is None:
        _BUILT = build_module()
    return _BUILT


def kernel(x, x_prev, w_q, w_kv, w_out, b_out):
    from concourse.bass_utils import run_bass_kernel_spmd

    nc = _get_module()
    in_maps = host_inputs(x, x_prev, w_q, w_kv, w_out, b_out)
    res = run_bass_kernel_spmd(nc, in_maps, core_ids=list(range(NCORES)))

    nq = N // 2
    out = np.empty((B, N, DIM), np.float32)
    for c in range(NCORES):
        b, half = c // 2, c % 2
        out[b, half * nq:(half + 1) * nq, :] = res.results[c]["out"]
    return out



# revision 28
# speedup vs baseline: 1.7585x; 1.7585x over previous
"""Multi-head attention (b=4, n=2048, dim=512, h=8, dh=64) on 8 trn2 cores.

Sharding: core c handles batch b=c//2 and query rows
[half*1024, (half+1)*1024) with half=c%2. K/V (from x_prev) are computed
redundantly on both of a batch's cores (cheap vs. attention). No collectives.

Per-core kernel (bf16 operands, fp32 PSUM accumulation), software-pipelined
so the Activation engine (exp over all nq*nk*h scores, ~137us busy) is fed
continuously while the PE interleaves projection / PV / out-proj matmuls
between score matmuls ("fillers"):

  QT[inner, nq]  = w_q-tiles  (lhsT) @ x^T          (q in transposed layout)
  KT[inner, nk]  = w_kv-tiles (lhsT) @ x_prev^T
  V [nk, inner]  = x_prev^T-tiles (lhsT) @ w_kv[:, v]  (+ ones column/head)
  ST[j, i]       = KT_h-tile (lhsT, K=dh=64) @ QT_h  (scores transposed;
                   two heads row-tiled in the PE at partitions 0/64)
  PT             = exp(ST * scale)    (no max subtraction: |s*scale| < ~8)
  PV[dh+1, i]    = sum_j V_h|ones (lhsT) @ PT_h      (row dh = sum_j P = l)
  l -> (dma to partitions 0/1) -> r = 1/l -> rb_h = DRAM-broadcast
  AOT[128, p]    = pv0*rb0 -> partitions 0:64, pv1*rb1 -> partitions 64:128
                   (head pair packed so out-proj runs at K=128)
  out[i, d]      = sum_p AOT_p-tiles (lhsT, K=128) @ wout_pair_p + bias_bcast
"""

import os
import numpy as np
import ml_dtypes

B, N, DIM = 4, 2048, 512
H, DH, INNER = 8, 64, 512
NCORES = 8

# debug ablations for sim experiments (comma-separated): "fakenorm"
_ABLATE = frozenset(os.environ.get("KERNEL_ABLATE", "").split(","))

_BUILT = None


def build_module(dim=DIM, h=H, nq=N // 2, nk=N, compile_module=True, reps=1):
    """Build the per-core attention module. dim % 128 == 0, h % 2 == 0,
    nq % 512 == 0, nk % 512 == 0. reps>1 repeats the whole compute body
    (timing calibration only)."""
    import concourse.mybir as mybir
    import concourse.tile as tile
    from concourse import bacc
    import contextlib

    CDT = mybir.dt.bfloat16
    FDT = mybir.dt.float32
    HDT = mybir.dt.float16
    Exp = mybir.ActivationFunctionType.Exp

    inner = h * DH
    nkt = dim // 128          # contraction tiles for projections
    npr = h // 2              # head pairs (= inner // 128 slices of KT/QT)
    nj = nk // 128            # key tiles
    nqc = nq // 512           # query chunks
    nrt = nq // 128           # output row tiles
    VW = DH + 1               # 65: per-head v columns + ones column
    scale = DH ** -0.5

    nc = bacc.Bacc("TRN2", target_bir_lowering=False, debug=False,
                   num_devices=NCORES)

    xt_d = nc.declare_dram_parameter("xt", [dim, nq], CDT, isOutput=False)
    xpt_d = nc.declare_dram_parameter("xpt", [dim, nk], CDT, isOutput=False)
    wq_d = nc.declare_dram_parameter("wq", [dim, inner], CDT, isOutput=False)
    wkv_d = nc.declare_dram_parameter("wkv", [dim, 2 * inner], CDT,
                                      isOutput=False)
    # w_out pre-arranged on host in head pairs:
    # wout[d, p, :] = w_out[(2p)*64 + d] for d<64, w_out[(2p+1)*64 + d-64] else
    wout_d = nc.declare_dram_parameter("wout", [128, npr, dim], CDT,
                                       isOutput=False)
    bout_d = nc.declare_dram_parameter("bout", [1, dim], FDT, isOutput=False)
    out_d = nc.declare_dram_parameter("out", [nq, dim], FDT, isOutput=True)
    # DRAM bounce rows for the 1/l partition-broadcast (SBUF APs cannot
    # have a zero-step partition dim; DRAM APs can).
    rsc_d = nc.dram_tensor("rscratch", [2 * npr * nqc, 512], FDT)

    with tile.TileContext(nc) as tc, contextlib.ExitStack() as stack:
        consts = stack.enter_context(tc.tile_pool(name="consts", bufs=1))
        acts = stack.enter_context(tc.tile_pool(name="acts", bufs=1))

        wq_sb = consts.tile([128, nkt, inner], CDT)
        wkv_sb = consts.tile([128, nkt, 2 * inner], CDT)
        wout_sb = consts.tile([128, npr, dim], CDT)
        bias_sb = consts.tile([128, dim], FDT)
        ones16 = consts.tile([1, 128], HDT)
        nc.vector.memset(ones16[:, :], 1.0)

        xt_sb = acts.tile([128, nkt, nq], CDT)
        xpt_sb = acts.tile([128, nkt, nk], CDT)

        # Input DMAs, column-chunked and spread across queues so the first
        # score matmul (needs kt s0 ch0 = wkv cols 0:128 + xpt cols 0:512,
        # and qt c0 = wq cols 0:128 + xt cols 0:512) can start at ~4us.
        wkv_in = wkv_d.ap().rearrange("(t p) o -> p t o", p=128)
        wq_in = wq_d.ap().rearrange("(t p) o -> p t o", p=128)
        xpt_in = xpt_d.ap().rearrange("(t p) n -> p t n", p=128)
        xt_in = xt_d.ap().rearrange("(t p) n -> p t n", p=128)
        nc.scalar.dma_start(out=wkv_sb[:, :, 0:128], in_=wkv_in[:, :, 0:128])
        nc.scalar.dma_start(out=wkv_sb[:, :, 128:2 * inner],
                            in_=wkv_in[:, :, 128:2 * inner])
        nc.sync.dma_start(out=xpt_sb[:, :, 0:128], in_=xpt_in[:, :, 0:128])
        nc.sync.dma_start(out=xt_sb[:, :, 0:512], in_=xt_in[:, :, 0:512])
        nc.sync.dma_start(out=xpt_sb[:, :, 128:512],
                          in_=xpt_in[:, :, 128:512])
        nc.sync.dma_start(out=xpt_sb[:, :, 512:1024],
                          in_=xpt_in[:, :, 512:1024])
        nc.gpsimd.dma_start(out=wq_sb[:, :, 0:128], in_=wq_in[:, :, 0:128])
        nc.gpsimd.dma_start(out=xpt_sb[:, :, 1024:1536],
                            in_=xpt_in[:, :, 1024:1536])
        nc.gpsimd.dma_start(out=xpt_sb[:, :, 1536:2048],
                            in_=xpt_in[:, :, 1536:2048])
        nc.gpsimd.dma_start(out=xt_sb[:, :, 512:nq], in_=xt_in[:, :, 512:nq])
        nc.gpsimd.dma_start(out=wq_sb[:, :, 128:inner],
                            in_=wq_in[:, :, 128:inner])
        nc.gpsimd.dma_start(out=wout_sb[:, :, :], in_=wout_d.ap())
        nc.gpsimd.dma_start(out=bias_sb[:, :],
                            in_=bout_d.ap().to_broadcast([128, dim]))

        qt_sb = acts.tile([128, npr, nq], CDT)    # [inner-slice, nq]
        kt_sb = acts.tile([128, npr, nk], CDT)    # [inner-slice, nk]
        v_sb = acts.tile([128, nj, h * VW], CDT)  # [key-tile, h*(dh+1)]
        aot_sb = acts.tile([128, npr, nq], CDT)   # head pair packed

        for hh in range(h):  # ones columns of V
            nc.vector.memset(v_sb[:, :, hh * VW + DH:hh * VW + DH + 1], 1.0)

        for _rep in range(reps):
            rep_stack = contextlib.ExitStack()
            st_ps = rep_stack.enter_context(
                tc.tile_pool(name="st_ps", bufs=2, space="PSUM"))
            acc_ps = rep_stack.enter_context(
                tc.tile_pool(name="acc_ps", bufs=4, space="PSUM"))
            pt_pool = rep_stack.enter_context(tc.tile_pool(name="pt", bufs=2))
            lr_pool = rep_stack.enter_context(tc.tile_pool(name="lr", bufs=3))

            # ---- filler units: (weight, closure) at single-matmul grain --
            # weight ~ PE cost in 512-column-matmul units. A projection
            # chunk is nkt accumulation matmuls sharing one psum tile
            # (allocated lazily in its first unit) + a trailing copy.
            def proj_units(kind, s, ch, c0=0, c1=512):
                box = []
                w = (c1 - c0) / 512.0

                def mm(k):
                    def go():
                        if k == 0:
                            box.append(acc_ps.tile([128, 512], FDT,
                                                   tag="acc", name="pp"))
                        ps = box[0]
                        if kind == "kt":
                            nc.tensor.matmul(
                                ps[:, 0:c1 - c0],
                                lhsT=wkv_sb[:, k, s * 128:(s + 1) * 128],
                                rhs=xpt_sb[:, k, ch * 512 + c0:ch * 512 + c1],
                                start=(k == 0), stop=(k == nkt - 1))
                        elif kind == "qt":
                            nc.tensor.matmul(
                                ps[:, :],
                                lhsT=wq_sb[:, k, s * 128:(s + 1) * 128],
                                rhs=xt_sb[:, k, ch * 512:(ch + 1) * 512],
                                start=(k == 0), stop=(k == nkt - 1))
                        else:  # v: s is the key tile index
                            nc.tensor.matmul(
                                ps[:, 0:inner],
                                lhsT=xpt_sb[:, k, s * 128:(s + 1) * 128],
                                rhs=wkv_sb[:, k, inner:2 * inner],
                                start=(k == 0), stop=(k == nkt - 1))
                        if k == nkt - 1:
                            if kind == "kt":
                                nc.vector.tensor_copy(
                                    out=kt_sb[:, s,
                                              ch * 512 + c0:ch * 512 + c1],
                                    in_=ps[:, 0:c1 - c0])
                            elif kind == "qt":
                                nc.vector.tensor_copy(
                                    out=qt_sb[:, s, ch * 512:(ch + 1) * 512],
                                    in_=ps[:, :])
                            else:
                                nc.vector.tensor_copy(
                                    out=v_sb[:, s, :].rearrange(
                                        "p (g x) -> p g x", x=VW)[:, :, 0:DH],
                                    in_=ps[:, 0:inner].rearrange(
                                        "p (g x) -> p g x", x=DH))
                    return go

                return [(w, mm(k)) for k in range(nkt)]

            def make_pv(c, p, pt):
                """Lazily allocate the pair's PV psum tiles (at first-unit
                emission, to keep acc-pool rotation in program order);
                return a per-(j, head) step closure."""
                box = []
                h0, h1 = 2 * p, 2 * p + 1

                def step(j, e):
                    if j == 0 and e == 0:
                        box.append(acc_ps.tile([128, 512], FDT, tag="acc",
                                               name="pv0"))
                        box.append(acc_ps.tile([128, 512], FDT, tag="acc",
                                               name="pv1"))
                    hh = h0 if e == 0 else h1
                    nc.tensor.matmul(
                        box[e][0:VW, :],
                        lhsT=v_sb[:, j, hh * VW:(hh + 1) * VW],
                        rhs=pt[:, j, e * 512:(e + 1) * 512],
                        start=(j == 0), stop=(j == nj - 1))

                return box, step

            def make_pvc(box):
                """Copy the pair's PV out of PSUM right away (DVE), so the
                acc-pool banks recycle without waiting for the norm chain's
                DRAM round trip."""
                sbc = []

                def go():
                    pvc0 = lr_pool.tile([VW, 512], FDT, tag="pvc0")
                    pvc1 = lr_pool.tile([VW, 512], FDT, tag="pvc1")
                    nc.vector.tensor_copy(out=pvc0[:, :], in_=box[0][0:VW, :])
                    nc.vector.tensor_copy(out=pvc1[:, :], in_=box[1][0:VW, :])
                    sbc.extend((pvc0, pvc1))
                return sbc, go

            def make_norm(c, p, sbc):
                """1/l via in-place DVE recip -> DRAM bounce broadcast ->
                Pool muls. Odd head lands at partitions 64:128 of aot
                (pair packing)."""
                idx = (c * npr + p) * 2

                def go():
                    pv0, pv1 = sbc
                    nc.vector.reciprocal(out=pv0[DH:DH + 1, :],
                                         in_=pv0[DH:DH + 1, :])
                    nc.vector.reciprocal(out=pv1[DH:DH + 1, :],
                                         in_=pv1[DH:DH + 1, :])
                    nc.sync.dma_start(out=rsc_d.ap()[idx:idx + 1, :],
                                      in_=pv0[DH:DH + 1, :])
                    nc.sync.dma_start(out=rsc_d.ap()[idx + 1:idx + 2, :],
                                      in_=pv1[DH:DH + 1, :])
                    rb0 = lr_pool.tile([DH, 512], FDT, tag="rb0")
                    rb1 = lr_pool.tile([DH, 512], FDT, tag="rb1")
                    if "fakenorm" in _ABLATE:
                        nc.gpsimd.memset(rb0[:, :], 1.0)
                        nc.gpsimd.memset(rb1[:, :], 1.0)
                    else:
                        nc.sync.dma_start(
                            out=rb0[:, :],
                            in_=rsc_d.ap()[idx:idx + 1, :].to_broadcast(
                                [DH, 512]))
                        nc.sync.dma_start(
                            out=rb1[:, :],
                            in_=rsc_d.ap()[idx + 1:idx + 2, :].to_broadcast(
                                [DH, 512]))
                    cc = slice(c * 512, (c + 1) * 512)
                    nc.gpsimd.tensor_mul(
                        aot_sb[0:DH, p, cc], pv0[0:DH, :], rb0[:, :])
                    nc.gpsimd.tensor_mul(
                        aot_sb[DH:128, p, cc], pv1[0:DH, :], rb1[:, :])
                return go

            def fout_units(t):
                box = []

                def mm(p):
                    def go():
                        if p == 0:
                            box.append(acc_ps.tile([128, 512], FDT,
                                                   tag="acc", name="f"))
                        nc.tensor.matmul(
                            box[0][:, :],
                            lhsT=aot_sb[:, p, t * 128:(t + 1) * 128],
                            rhs=wout_sb[:, p, :],
                            start=(p == 0), stop=(p == npr - 1))
                        if p == npr - 1:
                            fo = lr_pool.tile([128, dim], FDT, tag="fo")
                            nc.vector.tensor_add(out=fo[:, :],
                                                 in0=box[0][:, :],
                                                 in1=bias_sb[:, :])
                            nc.sync.dma_start(
                                out=out_d.ap()[t * 128:(t + 1) * 128, :],
                                in_=fo[:, :])
                    return go

                return [(1.0, mm(p)) for p in range(npr)]

            # ---- global filler queue: (weight, closure), FIFO, paced -----
            queue = []
            q_head = 0

            def drain(budget):
                nonlocal q_head
                spent = 0.0
                while q_head < len(queue) and spent < budget:
                    w, go = queue[q_head]
                    q_head += 1
                    go()
                    spent += w
                return spent

            def drain_all():
                nonlocal q_head
                while q_head < len(queue):
                    queue[q_head][1]()
                    q_head += 1

            # ---- phase: 16 S+exp slots, fillers drained between ----------
            def emit_pair(c, p, pt=None, inline_step=None):
                if pt is None:
                    pt = pt_pool.tile([128, nj, 1024], CDT, tag="pt")
                remaining = sum(w for w, _ in queue[q_head:])
                pace = remaining / nj
                spent = 0.0
                for j in range(nj):
                    st = st_ps.tile([128, 1024], FDT, tag="st")
                    nc.tensor.matmul(
                        st[:, 0:512],
                        lhsT=kt_sb[0:64, p, j * 128:(j + 1) * 128],
                        rhs=qt_sb[0:64, p, c * 512:(c + 1) * 512],
                        start=True, stop=True)
                    nc.tensor.matmul(
                        st[:, 512:1024],
                        lhsT=kt_sb[64:128, p, j * 128:(j + 1) * 128],
                        rhs=qt_sb[64:128, p, c * 512:(c + 1) * 512],
                        start=True, stop=True)
                    if "cheapexp" in _ABLATE:
                        nc.scalar.activation(out=pt[:, j, 0:64],
                                             in_=st[:, 0:64],
                                             func=Exp, scale=scale)
                    else:
                        nc.scalar.activation(out=pt[:, j, :], in_=st[:, :],
                                             func=Exp, scale=scale)
                    if inline_step is not None and j >= 2:
                        inline_step(j - 2, 0)
                        inline_step(j - 2, 1)
                    spent += drain(pace * (j + 1) - spent)
                return pt

            # ---- pre-phase: minimal kt (first j-tile) + qt c0 on the PE --
            for _, go in proj_units("kt", 0, 0, 0, 128):
                go()
            for _, go in proj_units("qt", 0, 0):
                go()

            # ---- pipelined pair phases -----------------------------------
            pair_list = [(c, p) for c in range(nqc) for p in range(npr)]
            carry = None  # (c, p, pt) of previous pair, PV not yet emitted
            for pi, (c, p) in enumerate(pair_list):
                last = pi == len(pair_list) - 1
                if pi == 0:
                    # rest of kt s0 (needed by this phase's own S slots,
                    # FIFO-ordered first) then next-pair projections and V
                    queue.extend(proj_units("kt", 0, 0, 128, 512))
                    for ch in range(1, nk // 512):
                        queue.extend(proj_units("kt", 0, ch))
                    queue.extend(proj_units("qt", 0, 1))
                    for ch in range(nk // 512):
                        queue.extend(proj_units("kt", 1, ch))
                    for ch in range(nqc):
                        queue.extend(proj_units("qt", 1, ch))
                    for j in range(nj):
                        queue.extend(proj_units("v", j, 0))
                elif pi < npr - 1:
                    for ch in range(nk // 512):
                        queue.extend(proj_units("kt", pi + 1, ch))
                    for ch in range(nqc):
                        queue.extend(proj_units("qt", pi + 1, ch))
                if carry is not None:
                    pc, pp, ppt = carry[:3]
                    box, step = make_pv(pc, pp, ppt)
                    for j in range(nj):
                        queue.append((1.0, lambda j=j, s=step: s(j, 0)))
                        queue.append((1.0, lambda j=j, s=step: s(j, 1)))
                    sbc, cgo = make_pvc(box)
                    queue.append((0.0, cgo))
                    queue.append((0.0, make_norm(pc, pp, sbc)))
                # out-proj of chunk 0 once all its pairs are normalized
                if nqc == 2:
                    if pi == npr + 1:
                        queue.extend(fout_units(0))
                        queue.extend(fout_units(1))
                    elif pi == npr + 2:
                        queue.extend(fout_units(2))
                        queue.extend(fout_units(3))

                if last:
                    # inline the final pair's PV into its own score phase
                    # (two slots behind the exp) so the tail stays short
                    pt = pt_pool.tile([128, nj, 1024], CDT, tag="pt")
                    box, step = make_pv(c, p, pt)
                    emit_pair(c, p, pt=pt, inline_step=step)
                    carry = (c, p, pt, box, step)
                else:
                    pt = emit_pair(c, p)
                    carry = (c, p, pt)

            # ---- tail: finish last PV; the final pair's 1/l broadcast goes
            # through the PE (ones outer-product) instead of the DRAM bounce
            # to shorten the chain, and only the out-proj's last accumulation
            # matmul waits on it.
            drain_all()
            c, p, pt, box, step = carry
            step(nj - 2, 0)
            step(nj - 2, 1)
            step(nj - 1, 0)
            step(nj - 1, 1)
            sbc, cgo = make_pvc(box)
            cgo()
            rr0 = lr_pool.tile([1, 512], HDT, tag="rr0")
            rr1 = lr_pool.tile([1, 512], HDT, tag="rr1")
            with nc.allow_low_precision(reason="1/l fits fp16 comfortably"):
                nc.vector.reciprocal(out=rr0[0:1, :],
                                     in_=sbc[0][DH:DH + 1, :])
                nc.vector.reciprocal(out=rr1[0:1, :],
                                     in_=sbc[1][DH:DH + 1, :])
            rb_ps = acc_ps.tile([128, 512], FDT, tag="acc", name="rbps")
            nc.tensor.matmul(rb_ps[0:DH, :], lhsT=ones16[0:1, 0:DH],
                             rhs=rr0[0:1, :], start=True, stop=True)
            nc.tensor.matmul(rb_ps[DH:128, :], lhsT=ones16[0:1, 0:DH],
                             rhs=rr1[0:1, :], start=True, stop=True)
            cc = slice(c * 512, (c + 1) * 512)
            nc.vector.tensor_mul(aot_sb[0:DH, p, cc], sbc[0][0:DH, :],
                                 rb_ps[0:DH, :])
            nc.vector.tensor_mul(aot_sb[DH:128, p, cc], sbc[1][0:DH, :],
                                 rb_ps[DH:128, :])
            tail_ts = list(range(4, nrt)) if nqc == 2 else list(range(nrt))
            fts = {t: fout_units(t) for t in tail_ts}
            for t in tail_ts:  # pairs 0..npr-2 don't need the last norm
                for w, go in fts[t][:npr - 1]:
                    go()
            for t in tail_ts:
                fts[t][npr - 1][1]()

            rep_stack.close()

    if compile_module:
        nc.compile()
    return nc


def host_inputs(x, x_prev, w_q, w_kv, w_out, b_out, ncores=NCORES):
    """Shard + lay out the full inputs into per-core input maps."""
    bf16 = ml_dtypes.bfloat16
    b, n, dim = x.shape
    inner = w_q.shape[1]
    h = inner // DH
    npr = h // 2
    nq = (b * n) // ncores
    halves = ncores // b
    wq = np.ascontiguousarray(w_q).astype(bf16)
    wkv = np.ascontiguousarray(w_kv).astype(bf16)
    wo = w_out.reshape(npr, 2 * DH, dim)
    wout = np.ascontiguousarray(wo.transpose(1, 0, 2)).astype(bf16)
    bout = np.ascontiguousarray(b_out).reshape(1, dim).astype(np.float32)
    in_maps = []
    for c in range(ncores):
        bb, half = c // halves, c % halves
        xt = np.ascontiguousarray(
            x[bb, half * nq:(half + 1) * nq, :].T).astype(bf16)
        xpt = np.ascontiguousarray(x_prev[bb].T).astype(bf16)
        in_maps.append(dict(xt=xt, xpt=xpt, wq=wq, wkv=wkv, wout=wout,
                            bout=bout))
    return in_maps


def _get_module():
    global _BUILT
    if _BUILT is None:
        _BUILT = build_module()
    return _BUILT


def kernel(x, x_prev, w_q, w_kv, w_out, b_out):
    from concourse.bass_utils import run_bass_kernel_spmd

    nc = _get_module()
    in_maps = host_inputs(x, x_prev, w_q, w_kv, w_out, b_out)
    res = run_bass_kernel_spmd(nc, in_maps, core_ids=list(range(NCORES)))

    nq = N // 2
    out = np.empty((B, N, DIM), np.float32)
    for c in range(NCORES):
        b, half = c // 2, c % 2
        out[b, half * nq:(half + 1) * nq, :] = res.results[c]["out"]
    return out


# revision 30
# speedup vs baseline: 2.0896x; 1.1883x over previous
"""Multi-head attention (b=4, n=2048, dim=512, h=8, dh=64) on 8 trn2 cores.

Sharding: core c handles batch b=c//2 and query rows
[half*1024, (half+1)*1024) with half=c%2. K/V (from x_prev) are computed
redundantly on both of a batch's cores (cheap vs. attention). No collectives.

Per-core kernel (bf16 operands, fp32 PSUM accumulation), software-pipelined
so the Activation engine (exp over all nq*nk*h scores, ~137us busy) is fed
continuously while the PE interleaves projection / PV / out-proj matmuls
between score matmuls ("fillers"):

  QT[inner, nq]  = w_q-tiles  (lhsT) @ x^T          (q in transposed layout)
  KT[inner, nk]  = w_kv-tiles (lhsT) @ x_prev^T
  V [nk, inner]  = x_prev^T-tiles (lhsT) @ w_kv[:, v]  (+ ones column/head)
  ST[j, i]       = KT_h-tile (lhsT, K=dh=64) @ QT_h  (scores transposed;
                   two heads row-tiled in the PE at partitions 0/64)
  PT             = exp(ST * scale)    (no max subtraction: |s*scale| < ~8)
  PV[dh+1, i]    = sum_j V_h|ones (lhsT) @ PT_h      (row dh = sum_j P = l)
  l -> (dma to partitions 0/1) -> r = 1/l -> rb_h = DRAM-broadcast
  AOT[128, p]    = pv0*rb0 -> partitions 0:64, pv1*rb1 -> partitions 64:128
                   (head pair packed so out-proj runs at K=128)
  out[i, d]      = sum_p AOT_p-tiles (lhsT, K=128) @ wout_pair_p + bias_bcast
"""

import os
import numpy as np
import ml_dtypes

B, N, DIM = 4, 2048, 512
H, DH, INNER = 8, 64, 512
NCORES = 8

# debug ablations for sim experiments (comma-separated): "fakenorm"
_ABLATE = frozenset(os.environ.get("KERNEL_ABLATE", "").split(","))

_BUILT = None


def build_module(dim=DIM, h=H, nq=N // 2, nk=N, compile_module=True, reps=1):
    """Build the per-core attention module. dim % 128 == 0, h % 2 == 0,
    nq % 512 == 0, nk % 512 == 0. reps>1 repeats the whole compute body
    (timing calibration only)."""
    import concourse.mybir as mybir
    import concourse.tile as tile
    from concourse import bacc
    import contextlib

    CDT = mybir.dt.bfloat16
    FDT = mybir.dt.float32
    HDT = mybir.dt.float16
    Exp = mybir.ActivationFunctionType.Exp

    inner = h * DH
    nkt = dim // 128          # contraction tiles for projections
    npr = h // 2              # head pairs (= inner // 128 slices of KT/QT)
    nj = nk // 128            # key tiles
    nqc = nq // 512           # query chunks
    nrt = nq // 128           # output row tiles
    VW = DH + 1               # 65: per-head v columns + ones column
    scale = DH ** -0.5

    nc = bacc.Bacc("TRN2", target_bir_lowering=False, debug=False,
                   num_devices=NCORES)

    xt_d = nc.declare_dram_parameter("xt", [dim, nq], CDT, isOutput=False)
    xpt_d = nc.declare_dram_parameter("xpt", [dim, nk], CDT, isOutput=False)
    wq_d = nc.declare_dram_parameter("wq", [dim, inner], CDT, isOutput=False)
    wkv_d = nc.declare_dram_parameter("wkv", [dim, 2 * inner], CDT,
                                      isOutput=False)
    # w_out pre-arranged on host in head pairs:
    # wout[d, p, :] = w_out[(2p)*64 + d] for d<64, w_out[(2p+1)*64 + d-64] else
    wout_d = nc.declare_dram_parameter("wout", [128, npr, dim], CDT,
                                       isOutput=False)
    bout_d = nc.declare_dram_parameter("bout", [1, dim], FDT, isOutput=False)
    out_d = nc.declare_dram_parameter("out", [nq, dim], FDT, isOutput=True)
    # DRAM bounce rows for the 1/l partition-broadcast (SBUF APs cannot
    # have a zero-step partition dim; DRAM APs can).
    rsc_d = nc.dram_tensor("rscratch", [2 * npr * nqc, 512],
                           mybir.dt.float16)

    with tile.TileContext(nc) as tc, contextlib.ExitStack() as stack:
        consts = stack.enter_context(tc.tile_pool(name="consts", bufs=1))
        acts = stack.enter_context(tc.tile_pool(name="acts", bufs=1))

        wq_sb = consts.tile([128, nkt, inner], CDT)
        wkv_sb = consts.tile([128, nkt, 2 * inner], CDT)
        wout_sb = consts.tile([128, npr, dim], CDT)
        bias_sb = consts.tile([128, dim], FDT)
        ones16 = consts.tile([1, 128], HDT)
        nc.vector.memset(ones16[:, :], 1.0)

        xt_sb = acts.tile([128, nkt, nq], CDT)
        xpt_sb = acts.tile([128, nkt, nk], CDT)

        # Input DMAs, column-chunked and spread across queues so the first
        # score matmul (needs kt s0 ch0 = wkv cols 0:128 + xpt cols 0:512,
        # and qt c0 = wq cols 0:128 + xt cols 0:512) can start at ~4us.
        wkv_in = wkv_d.ap().rearrange("(t p) o -> p t o", p=128)
        wq_in = wq_d.ap().rearrange("(t p) o -> p t o", p=128)
        xpt_in = xpt_d.ap().rearrange("(t p) n -> p t n", p=128)
        xt_in = xt_d.ap().rearrange("(t p) n -> p t n", p=128)
        nc.scalar.dma_start(out=wkv_sb[:, :, 0:128], in_=wkv_in[:, :, 0:128])
        nc.sync.dma_start(out=xpt_sb[:, :, 0:128], in_=xpt_in[:, :, 0:128])
        nc.sync.dma_start(out=xt_sb[:, :, 0:512], in_=xt_in[:, :, 0:512])
        nc.sync.dma_start(out=xpt_sb[:, :, 128:512],
                          in_=xpt_in[:, :, 128:512])
        nc.sync.dma_start(out=xpt_sb[:, :, 512:1024],
                          in_=xpt_in[:, :, 512:1024])
        nc.gpsimd.dma_start(out=wq_sb[:, :, 0:128], in_=wq_in[:, :, 0:128])
        nc.gpsimd.dma_start(out=wkv_sb[:, :, 128:2 * inner],
                            in_=wkv_in[:, :, 128:2 * inner])
        nc.gpsimd.dma_start(out=xpt_sb[:, :, 1024:1536],
                            in_=xpt_in[:, :, 1024:1536])
        nc.gpsimd.dma_start(out=xpt_sb[:, :, 1536:2048],
                            in_=xpt_in[:, :, 1536:2048])
        nc.gpsimd.dma_start(out=xt_sb[:, :, 512:nq], in_=xt_in[:, :, 512:nq])
        nc.gpsimd.dma_start(out=wq_sb[:, :, 128:inner],
                            in_=wq_in[:, :, 128:inner])
        nc.gpsimd.dma_start(out=wout_sb[:, :, :], in_=wout_d.ap())
        nc.gpsimd.dma_start(out=bias_sb[:, :],
                            in_=bout_d.ap().to_broadcast([128, dim]))

        qt_sb = acts.tile([128, npr, nq], CDT)    # [inner-slice, nq]
        kt_sb = acts.tile([128, npr, nk], CDT)    # [inner-slice, nk]
        v_sb = acts.tile([128, nj, h * VW], CDT)  # [key-tile, h*(dh+1)]
        aot_sb = acts.tile([128, npr, nq], CDT)   # head pair packed

        for hh in range(h):  # ones columns of V
            nc.vector.memset(v_sb[:, :, hh * VW + DH:hh * VW + DH + 1], 1.0)

        for _rep in range(reps):
            rep_stack = contextlib.ExitStack()
            st_ps = rep_stack.enter_context(
                tc.tile_pool(name="st_ps", bufs=2, space="PSUM"))
            acc_ps = rep_stack.enter_context(
                tc.tile_pool(name="acc_ps", bufs=4, space="PSUM"))
            pt_pool = rep_stack.enter_context(tc.tile_pool(name="pt", bufs=2))
            lr_pool = rep_stack.enter_context(tc.tile_pool(name="lr", bufs=3))

            # ---- filler units: (weight, closure) at single-matmul grain --
            # weight ~ PE cost in 512-column-matmul units. A projection
            # chunk is nkt accumulation matmuls sharing one psum tile
            # (allocated lazily in its first unit) + a trailing copy.
            def proj_units(kind, s, ch, c0=0, c1=512):
                box = []
                w = (c1 - c0) / 512.0

                def mm(k):
                    def go():
                        if k == 0:
                            box.append(acc_ps.tile([128, 512], FDT,
                                                   tag="acc", name="pp"))
                        ps = box[0]
                        if kind == "kt":
                            nc.tensor.matmul(
                                ps[:, 0:c1 - c0],
                                lhsT=wkv_sb[:, k, s * 128:(s + 1) * 128],
                                rhs=xpt_sb[:, k, ch * 512 + c0:ch * 512 + c1],
                                start=(k == 0), stop=(k == nkt - 1))
                        elif kind == "qt":
                            nc.tensor.matmul(
                                ps[:, :],
                                lhsT=wq_sb[:, k, s * 128:(s + 1) * 128],
                                rhs=xt_sb[:, k, ch * 512:(ch + 1) * 512],
                                start=(k == 0), stop=(k == nkt - 1))
                        else:  # v: s is the key tile index
                            nc.tensor.matmul(
                                ps[:, 0:inner],
                                lhsT=xpt_sb[:, k, s * 128:(s + 1) * 128],
                                rhs=wkv_sb[:, k, inner:2 * inner],
                                start=(k == 0), stop=(k == nkt - 1))
                        if k == nkt - 1:
                            if kind == "kt":
                                nc.vector.tensor_copy(
                                    out=kt_sb[:, s,
                                              ch * 512 + c0:ch * 512 + c1],
                                    in_=ps[:, 0:c1 - c0])
                            elif kind == "qt":
                                nc.vector.tensor_copy(
                                    out=qt_sb[:, s, ch * 512:(ch + 1) * 512],
                                    in_=ps[:, :])
                            else:
                                nc.vector.tensor_copy(
                                    out=v_sb[:, s, :].rearrange(
                                        "p (g x) -> p g x", x=VW)[:, :, 0:DH],
                                    in_=ps[:, 0:inner].rearrange(
                                        "p (g x) -> p g x", x=DH))
                    return go

                return [(w, mm(k)) for k in range(nkt)]

            def make_pv(c, p, pt):
                """Lazily allocate the pair's PV psum tiles (at first-unit
                emission, to keep acc-pool rotation in program order);
                return a per-(j, head) step closure."""
                box = []
                h0, h1 = 2 * p, 2 * p + 1

                def step(j, e):
                    if j == 0 and e == 0:
                        box.append(acc_ps.tile([128, 512], FDT, tag="acc",
                                               name="pv0"))
                        box.append(acc_ps.tile([128, 512], FDT, tag="acc",
                                               name="pv1"))
                    hh = h0 if e == 0 else h1
                    nc.tensor.matmul(
                        box[e][0:VW, :],
                        lhsT=v_sb[:, j, hh * VW:(hh + 1) * VW],
                        rhs=pt[:, j, e * 512:(e + 1) * 512],
                        start=(j == 0), stop=(j == nj - 1))

                return box, step

            def make_pvc(box):
                """Copy the pair's PV out of PSUM right away (DVE), so the
                acc-pool banks recycle without waiting for the norm chain's
                DRAM round trip."""
                sbc = []

                def go():
                    pvc0 = lr_pool.tile([VW, 512], FDT, tag="pvc0")
                    pvc1 = lr_pool.tile([VW, 512], FDT, tag="pvc1")
                    nc.vector.tensor_copy(out=pvc0[:, :], in_=box[0][0:VW, :])
                    nc.vector.tensor_copy(out=pvc1[:, :], in_=box[1][0:VW, :])
                    sbc.extend((pvc0, pvc1))
                return sbc, go

            def make_norm(c, p, sbc):
                """1/l via DVE recip (fp16 rows) -> DRAM bounce broadcast ->
                Pool muls. Odd head lands at partitions 64:128 of aot
                (pair packing)."""
                idx = (c * npr + p) * 2

                def go():
                    pv0, pv1 = sbc
                    rr0 = lr_pool.tile([1, 512], HDT, tag="rr0")
                    rr1 = lr_pool.tile([1, 512], HDT, tag="rr1")
                    with nc.allow_low_precision(reason="1/l fits fp16"):
                        nc.vector.reciprocal(out=rr0[0:1, :],
                                             in_=pv0[DH:DH + 1, :])
                        nc.vector.reciprocal(out=rr1[0:1, :],
                                             in_=pv1[DH:DH + 1, :])
                    nc.sync.dma_start(out=rsc_d.ap()[idx:idx + 1, :],
                                      in_=rr0[0:1, :])
                    nc.sync.dma_start(out=rsc_d.ap()[idx + 1:idx + 2, :],
                                      in_=rr1[0:1, :])
                    rb0 = lr_pool.tile([DH, 512], HDT, tag="rb0")
                    rb1 = lr_pool.tile([DH, 512], HDT, tag="rb1")
                    if "fakenorm" in _ABLATE:
                        nc.gpsimd.memset(rb0[:, :], 1.0)
                        nc.gpsimd.memset(rb1[:, :], 1.0)
                    else:
                        nc.sync.dma_start(
                            out=rb0[:, :],
                            in_=rsc_d.ap()[idx:idx + 1, :].to_broadcast(
                                [DH, 512]))
                        nc.sync.dma_start(
                            out=rb1[:, :],
                            in_=rsc_d.ap()[idx + 1:idx + 2, :].to_broadcast(
                                [DH, 512]))
                    cc = slice(c * 512, (c + 1) * 512)
                    nc.gpsimd.tensor_mul(
                        aot_sb[0:DH, p, cc], pv0[0:DH, :], rb0[:, :])
                    nc.gpsimd.tensor_mul(
                        aot_sb[DH:128, p, cc], pv1[0:DH, :], rb1[:, :])
                return go

            def fout_units(t):
                box = []

                def mm(p):
                    def go():
                        if p == 0:
                            box.append(acc_ps.tile([128, 512], FDT,
                                                   tag="acc", name="f"))
                        nc.tensor.matmul(
                            box[0][:, :],
                            lhsT=aot_sb[:, p, t * 128:(t + 1) * 128],
                            rhs=wout_sb[:, p, :],
                            start=(p == 0), stop=(p == npr - 1))
                        if p == npr - 1:
                            fo = lr_pool.tile([128, dim], FDT, tag="fo")
                            nc.vector.tensor_add(out=fo[:, :],
                                                 in0=box[0][:, :],
                                                 in1=bias_sb[:, :])
                            nc.sync.dma_start(
                                out=out_d.ap()[t * 128:(t + 1) * 128, :],
                                in_=fo[:, :])
                    return go

                return [(1.0, mm(p)) for p in range(npr)]

            # ---- global filler queue: (weight, closure), FIFO, paced -----
            queue = []
            q_head = 0

            def drain(budget):
                nonlocal q_head
                spent = 0.0
                while q_head < len(queue) and spent < budget:
                    w, go = queue[q_head]
                    q_head += 1
                    go()
                    spent += w
                return spent

            def drain_all():
                nonlocal q_head
                while q_head < len(queue):
                    queue[q_head][1]()
                    q_head += 1

            # ---- phase: 16 S+exp slots, fillers drained between ----------
            def emit_pair(c, p, pt=None, inline_step=None):
                if pt is None:
                    pt = pt_pool.tile([128, nj, 1024], CDT, tag="pt")
                remaining = sum(w for w, _ in queue[q_head:])
                pace = remaining / nj
                spent = 0.0
                for j in range(nj):
                    st = st_ps.tile([128, 1024], FDT, tag="st")
                    nc.tensor.matmul(
                        st[:, 0:512],
                        lhsT=kt_sb[0:64, p, j * 128:(j + 1) * 128],
                        rhs=qt_sb[0:64, p, c * 512:(c + 1) * 512],
                        start=True, stop=True)
                    nc.tensor.matmul(
                        st[:, 512:1024],
                        lhsT=kt_sb[64:128, p, j * 128:(j + 1) * 128],
                        rhs=qt_sb[64:128, p, c * 512:(c + 1) * 512],
                        start=True, stop=True)
                    if "cheapexp" in _ABLATE:
                        nc.scalar.activation(out=pt[:, j, 0:64],
                                             in_=st[:, 0:64],
                                             func=Exp, scale=scale)
                    else:
                        nc.scalar.activation(out=pt[:, j, :], in_=st[:, :],
                                             func=Exp, scale=scale)
                    if inline_step is not None and j >= 2:
                        inline_step(j - 2, 0)
                        inline_step(j - 2, 1)
                    spent += drain(pace * (j + 1) - spent)
                return pt

            # ---- pre-phase: minimal kt (first j-tile) + qt c0 on the PE --
            for _, go in proj_units("kt", 0, 0, 0, 128):
                go()
            for _, go in proj_units("qt", 0, 0):
                go()

            # ---- pipelined pair phases -----------------------------------
            pair_list = [(c, p) for c in range(nqc) for p in range(npr)]
            carry = None  # (c, p, pt) of previous pair, PV not yet emitted
            for pi, (c, p) in enumerate(pair_list):
                last = pi == len(pair_list) - 1
                if pi == 0:
                    # rest of kt s0 (needed by this phase's own S slots,
                    # FIFO-ordered first) then next-pair projections and V
                    queue.extend(proj_units("kt", 0, 0, 128, 512))
                    for ch in range(1, nk // 512):
                        queue.extend(proj_units("kt", 0, ch))
                    queue.extend(proj_units("qt", 0, 1))
                    for ch in range(nk // 512):
                        queue.extend(proj_units("kt", 1, ch))
                    for ch in range(nqc):
                        queue.extend(proj_units("qt", 1, ch))
                    for j in range(nj):
                        queue.extend(proj_units("v", j, 0))
                elif pi < npr - 1:
                    for ch in range(nk // 512):
                        queue.extend(proj_units("kt", pi + 1, ch))
                    for ch in range(nqc):
                        queue.extend(proj_units("qt", pi + 1, ch))
                if carry is not None:
                    pc, pp, ppt = carry[:3]
                    box, step = make_pv(pc, pp, ppt)
                    for j in range(nj):
                        queue.append((1.0, lambda j=j, s=step: s(j, 0)))
                        queue.append((1.0, lambda j=j, s=step: s(j, 1)))
                    sbc, cgo = make_pvc(box)
                    queue.append((0.0, cgo))
                    queue.append((0.0, make_norm(pc, pp, sbc)))
                # out-proj of chunk 0 once all its pairs are normalized
                if nqc == 2:
                    if pi == npr + 1:
                        queue.extend(fout_units(0))
                        queue.extend(fout_units(1))
                    elif pi == npr + 2:
                        queue.extend(fout_units(2))
                        queue.extend(fout_units(3))

                if last:
                    # inline the final pair's PV into its own score phase
                    # (two slots behind the exp) so the tail stays short
                    pt = pt_pool.tile([128, nj, 1024], CDT, tag="pt")
                    box, step = make_pv(c, p, pt)
                    emit_pair(c, p, pt=pt, inline_step=step)
                    carry = (c, p, pt, box, step)
                else:
                    pt = emit_pair(c, p)
                    carry = (c, p, pt)

            # ---- tail: finish last PV; the final pair's 1/l broadcast goes
            # through the PE (ones outer-product) instead of the DRAM bounce
            # to shorten the chain, and only the out-proj's last accumulation
            # matmul waits on it.
            drain_all()
            c, p, pt, box, step = carry
            step(nj - 2, 0)
            step(nj - 2, 1)
            step(nj - 1, 0)
            step(nj - 1, 1)
            sbc, cgo = make_pvc(box)
            cgo()
            rr0 = lr_pool.tile([1, 512], HDT, tag="rr0")
            rr1 = lr_pool.tile([1, 512], HDT, tag="rr1")
            with nc.allow_low_precision(reason="1/l fits fp16 comfortably"):
                nc.vector.reciprocal(out=rr0[0:1, :],
                                     in_=sbc[0][DH:DH + 1, :])
                nc.vector.reciprocal(out=rr1[0:1, :],
                                     in_=sbc[1][DH:DH + 1, :])
            rb_ps = acc_ps.tile([128, 512], FDT, tag="acc", name="rbps")
            nc.tensor.matmul(rb_ps[0:DH, :], lhsT=ones16[0:1, 0:DH],
                             rhs=rr0[0:1, :], start=True, stop=True)
            nc.tensor.matmul(rb_ps[DH:128, :], lhsT=ones16[0:1, 0:DH],
                             rhs=rr1[0:1, :], start=True, stop=True)
            cc = slice(c * 512, (c + 1) * 512)
            nc.vector.tensor_mul(aot_sb[0:DH, p, cc], sbc[0][0:DH, :],
                                 rb_ps[0:DH, :])
            nc.vector.tensor_mul(aot_sb[DH:128, p, cc], sbc[1][0:DH, :],
                                 rb_ps[DH:128, :])
            tail_ts = list(range(4, nrt)) if nqc == 2 else list(range(nrt))
            fts = {t: fout_units(t) for t in tail_ts}
            for t in tail_ts:  # pairs 0..npr-2 don't need the last norm
                for w, go in fts[t][:npr - 1]:
                    go()
            for t in tail_ts:
                fts[t][npr - 1][1]()

            rep_stack.close()

    if compile_module:
        nc.compile()
    return nc


def host_inputs(x, x_prev, w_q, w_kv, w_out, b_out, ncores=NCORES):
    """Shard + lay out the full inputs into per-core input maps."""
    bf16 = ml_dtypes.bfloat16
    b, n, dim = x.shape
    inner = w_q.shape[1]
    h = inner // DH
    npr = h // 2
    nq = (b * n) // ncores
    halves = ncores // b
    wq = np.ascontiguousarray(w_q).astype(bf16)
    wkv = np.ascontiguousarray(w_kv).astype(bf16)
    wo = w_out.reshape(npr, 2 * DH, dim)
    wout = np.ascontiguousarray(wo.transpose(1, 0, 2)).astype(bf16)
    bout = np.ascontiguousarray(b_out).reshape(1, dim).astype(np.float32)
    in_maps = []
    for c in range(ncores):
        bb, half = c // halves, c % halves
        xt = np.ascontiguousarray(
            x[bb, half * nq:(half + 1) * nq, :].T).astype(bf16)
        xpt = np.ascontiguousarray(x_prev[bb].T).astype(bf16)
        in_maps.append(dict(xt=xt, xpt=xpt, wq=wq, wkv=wkv, wout=wout,
                            bout=bout))
    return in_maps


def _get_module():
    global _BUILT
    if _BUILT is None:
        _BUILT = build_module()
    return _BUILT


def kernel(x, x_prev, w_q, w_kv, w_out, b_out):
    from concourse.bass_utils import run_bass_kernel_spmd

    nc = _get_module()
    in_maps = host_inputs(x, x_prev, w_q, w_kv, w_out, b_out)
    res = run_bass_kernel_spmd(nc, in_maps, core_ids=list(range(NCORES)))

    nq = N // 2
    out = np.empty((B, N, DIM), np.float32)
    for c in range(NCORES):
        b, half = c // 2, c % 2
        out[b, half * nq:(half + 1) * nq, :] = res.results[c]["out"]
    return out


# revision 33
# speedup vs baseline: 2.1853x; 1.0458x over previous
"""Multi-head attention (b=4, n=2048, dim=512, h=8, dh=64) on 8 trn2 cores.

Sharding: core c handles batch b=c//2 and query rows
[half*1024, (half+1)*1024) with half=c%2. K/V (from x_prev) are computed
redundantly on both of a batch's cores (cheap vs. attention). No collectives.

Per-core kernel (bf16 operands, fp32 PSUM accumulation), software-pipelined
so the Activation engine (exp over all nq*nk*h scores, ~137us busy) is fed
continuously while the PE interleaves projection / PV / out-proj matmuls
between score matmuls ("fillers"):

  QT[inner, nq]  = w_q-tiles  (lhsT) @ x^T          (q in transposed layout)
  KT[inner, nk]  = w_kv-tiles (lhsT) @ x_prev^T
  V [nk, inner]  = x_prev^T-tiles (lhsT) @ w_kv[:, v]  (+ ones column/head)
  ST[j, i]       = KT_h-tile (lhsT, K=dh=64) @ QT_h  (scores transposed;
                   two heads row-tiled in the PE at partitions 0/64)
  PT             = exp(ST * scale)    (no max subtraction: |s*scale| < ~8)
  PV[dh+1, i]    = sum_j V_h|ones (lhsT) @ PT_h      (row dh = sum_j P = l)
  l -> (dma to partitions 0/1) -> r = 1/l -> rb_h = DRAM-broadcast
  AOT[128, p]    = pv0*rb0 -> partitions 0:64, pv1*rb1 -> partitions 64:128
                   (head pair packed so out-proj runs at K=128)
  out[i, d]      = sum_p AOT_p-tiles (lhsT, K=128) @ wout_pair_p + bias_bcast
"""

import os
import numpy as np
import ml_dtypes

B, N, DIM = 4, 2048, 512
H, DH, INNER = 8, 64, 512
NCORES = 8

# debug ablations for sim experiments (comma-separated): "fakenorm"
_ABLATE = frozenset(os.environ.get("KERNEL_ABLATE", "").split(","))

_BUILT = None


def build_module(dim=DIM, h=H, nq=N // 2, nk=N, compile_module=True, reps=1):
    """Build the per-core attention module. dim % 128 == 0, h % 2 == 0,
    nq % 512 == 0, nk % 512 == 0. reps>1 repeats the whole compute body
    (timing calibration only)."""
    import concourse.mybir as mybir
    import concourse.tile as tile
    from concourse import bacc
    import contextlib

    CDT = mybir.dt.bfloat16
    FDT = mybir.dt.float32
    HDT = mybir.dt.float16
    Exp = mybir.ActivationFunctionType.Exp

    inner = h * DH
    nkt = dim // 128          # contraction tiles for projections
    npr = h // 2              # head pairs (= inner // 128 slices of KT/QT)
    nj = nk // 128            # key tiles
    nqc = nq // 512           # query chunks
    nrt = nq // 128           # output row tiles
    VW = DH + 1               # 65: per-head v columns + ones column
    scale = DH ** -0.5

    nc = bacc.Bacc("TRN2", target_bir_lowering=False, debug=False,
                   num_devices=NCORES)

    xt_d = nc.declare_dram_parameter("xt", [dim, nq], CDT, isOutput=False)
    xpt_d = nc.declare_dram_parameter("xpt", [dim, nk], CDT, isOutput=False)
    wq_d = nc.declare_dram_parameter("wq", [dim, inner], CDT, isOutput=False)
    wkv_d = nc.declare_dram_parameter("wkv", [dim, 2 * inner], CDT,
                                      isOutput=False)
    # w_out pre-arranged on host in head pairs:
    # wout[d, p, :] = w_out[(2p)*64 + d] for d<64, w_out[(2p+1)*64 + d-64] else
    wout_d = nc.declare_dram_parameter("wout", [128, npr, dim], CDT,
                                       isOutput=False)
    bout_d = nc.declare_dram_parameter("bout", [1, dim], CDT, isOutput=False)
    out_d = nc.declare_dram_parameter("out", [nq, dim], FDT, isOutput=True)
    # DRAM bounce rows for the 1/l partition-broadcast (SBUF APs cannot
    # have a zero-step partition dim; DRAM APs can).
    rsc_d = nc.dram_tensor("rscratch", [2 * npr * nqc, 512],
                           mybir.dt.float16)

    with tile.TileContext(nc) as tc, contextlib.ExitStack() as stack:
        consts = stack.enter_context(tc.tile_pool(name="consts", bufs=1))
        acts = stack.enter_context(tc.tile_pool(name="acts", bufs=1))

        wq_sb = consts.tile([128, nkt, inner], CDT)
        wkv_sb = consts.tile([128, nkt, 2 * inner], CDT)
        wout_sb = consts.tile([128, npr, dim], CDT)
        brow = consts.tile([1, dim], CDT)
        ones16 = consts.tile([1, 128], HDT)
        onesb = consts.tile([1, 128], CDT)
        nc.vector.memset(ones16[:, :], 1.0)
        nc.vector.memset(onesb[:, :], 1.0)

        xt_sb = acts.tile([128, nkt, nq], CDT)
        xpt_sb = acts.tile([128, nkt, nk], CDT)

        # Input DMAs, column-chunked and spread across queues so the first
        # score matmul (needs kt s0 ch0 = wkv cols 0:128 + xpt cols 0:512,
        # and qt c0 = wq cols 0:128 + xt cols 0:512) can start at ~4us.
        wkv_in = wkv_d.ap().rearrange("(t p) o -> p t o", p=128)
        wq_in = wq_d.ap().rearrange("(t p) o -> p t o", p=128)
        xpt_in = xpt_d.ap().rearrange("(t p) n -> p t n", p=128)
        xt_in = xt_d.ap().rearrange("(t p) n -> p t n", p=128)
        nc.scalar.dma_start(out=wkv_sb[:, :, 0:128], in_=wkv_in[:, :, 0:128])
        nc.sync.dma_start(out=xpt_sb[:, :, 0:128], in_=xpt_in[:, :, 0:128])
        nc.sync.dma_start(out=xt_sb[:, :, 0:512], in_=xt_in[:, :, 0:512])
        nc.sync.dma_start(out=xpt_sb[:, :, 128:512],
                          in_=xpt_in[:, :, 128:512])
        nc.sync.dma_start(out=xpt_sb[:, :, 512:1024],
                          in_=xpt_in[:, :, 512:1024])
        nc.gpsimd.dma_start(out=wq_sb[:, :, 0:128], in_=wq_in[:, :, 0:128])
        nc.gpsimd.dma_start(out=wkv_sb[:, :, 128:2 * inner],
                            in_=wkv_in[:, :, 128:2 * inner])
        nc.gpsimd.dma_start(out=xpt_sb[:, :, 1024:1536],
                            in_=xpt_in[:, :, 1024:1536])
        nc.gpsimd.dma_start(out=xpt_sb[:, :, 1536:2048],
                            in_=xpt_in[:, :, 1536:2048])
        nc.gpsimd.dma_start(out=xt_sb[:, :, 512:nq], in_=xt_in[:, :, 512:nq])
        nc.gpsimd.dma_start(out=wq_sb[:, :, 128:inner],
                            in_=wq_in[:, :, 128:inner])
        nc.gpsimd.dma_start(out=wout_sb[:, :, :], in_=wout_d.ap())
        nc.gpsimd.dma_start(out=brow[:, :], in_=bout_d.ap())

        qt_sb = acts.tile([128, npr, nq], CDT)    # [inner-slice, nq]
        kt_sb = acts.tile([128, npr, nk], CDT)    # [inner-slice, nk]
        v_sb = acts.tile([128, nj, h * VW], CDT)  # [key-tile, h*(dh+1)]
        aot_sb = acts.tile([128, npr, nq], CDT)   # head pair packed

        for hh in range(h):  # ones columns of V
            nc.vector.memset(v_sb[:, :, hh * VW + DH:hh * VW + DH + 1], 1.0)

        for _rep in range(reps):
            rep_stack = contextlib.ExitStack()
            st_ps = rep_stack.enter_context(
                tc.tile_pool(name="st_ps", bufs=2, space="PSUM"))
            acc_ps = rep_stack.enter_context(
                tc.tile_pool(name="acc_ps", bufs=4, space="PSUM"))
            pt_pool = rep_stack.enter_context(tc.tile_pool(name="pt", bufs=2))
            lr_pool = rep_stack.enter_context(tc.tile_pool(name="lr", bufs=3))

            # ---- filler units: (weight, closure) at single-matmul grain --
            # weight ~ PE cost in 512-column-matmul units. A projection
            # chunk is nkt accumulation matmuls sharing one psum tile
            # (allocated lazily in its first unit) + a trailing copy.
            def proj_units(kind, s, ch, c0=0, c1=512):
                box = []
                w = (c1 - c0) / 512.0

                def mm(k):
                    def go():
                        if k == 0:
                            box.append(acc_ps.tile([128, 512], FDT,
                                                   tag="acc", name="pp"))
                        ps = box[0]
                        if kind == "kt":
                            nc.tensor.matmul(
                                ps[:, 0:c1 - c0],
                                lhsT=wkv_sb[:, k, s * 128:(s + 1) * 128],
                                rhs=xpt_sb[:, k, ch * 512 + c0:ch * 512 + c1],
                                start=(k == 0), stop=(k == nkt - 1))
                        elif kind == "qt":
                            nc.tensor.matmul(
                                ps[:, :],
                                lhsT=wq_sb[:, k, s * 128:(s + 1) * 128],
                                rhs=xt_sb[:, k, ch * 512:(ch + 1) * 512],
                                start=(k == 0), stop=(k == nkt - 1))
                        else:  # v: s is the key tile index
                            nc.tensor.matmul(
                                ps[:, 0:inner],
                                lhsT=xpt_sb[:, k, s * 128:(s + 1) * 128],
                                rhs=wkv_sb[:, k, inner:2 * inner],
                                start=(k == 0), stop=(k == nkt - 1))
                        if k == nkt - 1:
                            if kind == "kt":
                                nc.vector.tensor_copy(
                                    out=kt_sb[:, s,
                                              ch * 512 + c0:ch * 512 + c1],
                                    in_=ps[:, 0:c1 - c0])
                            elif kind == "qt":
                                nc.vector.tensor_copy(
                                    out=qt_sb[:, s, ch * 512:(ch + 1) * 512],
                                    in_=ps[:, :])
                            else:
                                nc.vector.tensor_copy(
                                    out=v_sb[:, s, :].rearrange(
                                        "p (g x) -> p g x", x=VW)[:, :, 0:DH],
                                    in_=ps[:, 0:inner].rearrange(
                                        "p (g x) -> p g x", x=DH))
                    return go

                return [(w, mm(k)) for k in range(nkt)]

            def make_pv(c, p, pt):
                """Lazily allocate the pair's PV psum tiles (at first-unit
                emission, to keep acc-pool rotation in program order);
                return a per-(j, head) step closure."""
                box = []
                h0, h1 = 2 * p, 2 * p + 1

                def step(j, e):
                    if j == 0 and e == 0:
                        box.append(acc_ps.tile([128, 512], FDT, tag="acc",
                                               name="pv0"))
                        box.append(acc_ps.tile([128, 512], FDT, tag="acc",
                                               name="pv1"))
                    hh = h0 if e == 0 else h1
                    nc.tensor.matmul(
                        box[e][0:VW, :],
                        lhsT=v_sb[:, j, hh * VW:(hh + 1) * VW],
                        rhs=pt[:, j, e * 512:(e + 1) * 512],
                        start=(j == 0), stop=(j == nj - 1))

                return box, step

            def make_pvc(box):
                """Copy the pair's PV out of PSUM right away (DVE), so the
                acc-pool banks recycle without waiting for the norm chain's
                DRAM round trip."""
                sbc = []

                def go():
                    pvc0 = lr_pool.tile([VW, 512], FDT, tag="pvc0")
                    pvc1 = lr_pool.tile([VW, 512], FDT, tag="pvc1")
                    nc.vector.tensor_copy(out=pvc0[:, :], in_=box[0][0:VW, :])
                    nc.vector.tensor_copy(out=pvc1[:, :], in_=box[1][0:VW, :])
                    sbc.extend((pvc0, pvc1))
                return sbc, go

            def make_norm(c, p, sbc):
                """1/l via DVE recip (fp16 rows) -> DRAM bounce broadcast ->
                Pool muls. Odd head lands at partitions 64:128 of aot
                (pair packing)."""
                idx = (c * npr + p) * 2

                def go():
                    pv0, pv1 = sbc
                    rr0 = lr_pool.tile([1, 512], HDT, tag="rr0")
                    rr1 = lr_pool.tile([1, 512], HDT, tag="rr1")
                    with nc.allow_low_precision(reason="1/l fits fp16"):
                        nc.vector.reciprocal(out=rr0[0:1, :],
                                             in_=pv0[DH:DH + 1, :])
                        nc.vector.reciprocal(out=rr1[0:1, :],
                                             in_=pv1[DH:DH + 1, :])
                    nc.sync.dma_start(out=rsc_d.ap()[idx:idx + 1, :],
                                      in_=rr0[0:1, :])
                    nc.sync.dma_start(out=rsc_d.ap()[idx + 1:idx + 2, :],
                                      in_=rr1[0:1, :])
                    rb0 = lr_pool.tile([DH, 512], HDT, tag="rb0")
                    rb1 = lr_pool.tile([DH, 512], HDT, tag="rb1")
                    if "fakenorm" in _ABLATE:
                        nc.gpsimd.memset(rb0[:, :], 1.0)
                        nc.gpsimd.memset(rb1[:, :], 1.0)
                    else:
                        nc.sync.dma_start(
                            out=rb0[:, :],
                            in_=rsc_d.ap()[idx:idx + 1, :].to_broadcast(
                                [DH, 512]))
                        nc.sync.dma_start(
                            out=rb1[:, :],
                            in_=rsc_d.ap()[idx + 1:idx + 2, :].to_broadcast(
                                [DH, 512]))
                    cc = slice(c * 512, (c + 1) * 512)
                    nc.gpsimd.tensor_mul(
                        aot_sb[0:DH, p, cc], pv0[0:DH, :], rb0[:, :])
                    nc.gpsimd.tensor_mul(
                        aot_sb[DH:128, p, cc], pv1[0:DH, :], rb1[:, :])
                return go

            def fout_units(t, tail=False):
                box = []

                def bias_mm():
                    box.append(acc_ps.tile([128, 512], FDT, tag="acc",
                                           name="f"))
                    nc.tensor.matmul(box[0][:, :], lhsT=onesb[0:1, :],
                                     rhs=brow[0:1, :], start=True, stop=False)

                def mm(p):
                    def go():
                        nc.tensor.matmul(
                            box[0][:, :],
                            lhsT=aot_sb[:, p, t * 128:(t + 1) * 128],
                            rhs=wout_sb[:, p, :],
                            start=False, stop=(p == npr - 1))
                        if p == npr - 1:
                            fo = lr_pool.tile([128, dim], FDT, tag="fo")
                            if tail and t % 2 == 1:
                                nc.scalar.activation(
                                    out=fo[:, :], in_=box[0][:, :],
                                    func=mybir.ActivationFunctionType.Copy)
                            else:
                                nc.vector.tensor_copy(out=fo[:, :],
                                                      in_=box[0][:, :])
                            q = nc.sync if t % 2 == 0 else nc.gpsimd
                            q.dma_start(
                                out=out_d.ap()[t * 128:(t + 1) * 128, :],
                                in_=fo[:, :])
                    return go

                return ([(0.25, bias_mm)] +
                        [(1.0, mm(p)) for p in range(npr)])

            # ---- global filler queue: (weight, closure), FIFO, paced -----
            queue = []
            q_head = 0

            def drain(budget):
                nonlocal q_head
                spent = 0.0
                while q_head < len(queue) and spent < budget:
                    w, go = queue[q_head]
                    q_head += 1
                    go()
                    spent += w
                return spent

            def drain_all():
                nonlocal q_head
                while q_head < len(queue):
                    queue[q_head][1]()
                    q_head += 1

            # ---- phase: 16 S+exp slots, fillers drained between ----------
            def emit_pair(c, p, pt=None, inline_step=None):
                if pt is None:
                    pt = pt_pool.tile([128, nj, 1024], CDT, tag="pt")
                remaining = sum(w for w, _ in queue[q_head:])
                pace = remaining / nj
                spent = 0.0
                for j in range(nj):
                    st = st_ps.tile([128, 1024], FDT, tag="st")
                    nc.tensor.matmul(
                        st[:, 0:512],
                        lhsT=kt_sb[0:64, p, j * 128:(j + 1) * 128],
                        rhs=qt_sb[0:64, p, c * 512:(c + 1) * 512],
                        start=True, stop=True)
                    nc.tensor.matmul(
                        st[:, 512:1024],
                        lhsT=kt_sb[64:128, p, j * 128:(j + 1) * 128],
                        rhs=qt_sb[64:128, p, c * 512:(c + 1) * 512],
                        start=True, stop=True)
                    if "cheapexp" in _ABLATE:
                        nc.scalar.activation(out=pt[:, j, 0:64],
                                             in_=st[:, 0:64],
                                             func=Exp, scale=scale)
                    else:
                        nc.scalar.activation(out=pt[:, j, :], in_=st[:, :],
                                             func=Exp, scale=scale)
                    if inline_step is not None and j >= 2:
                        inline_step(j - 2, 0)
                        inline_step(j - 2, 1)
                    spent += drain(pace * (j + 1) - spent)
                return pt

            # ---- pre-phase: minimal kt (first j-tile) + qt c0 on the PE --
            for _, go in proj_units("kt", 0, 0, 0, 128):
                go()
            for _, go in proj_units("qt", 0, 0):
                go()

            # ---- pipelined pair phases -----------------------------------
            # chunk-0 pairs: PV runs as fillers of the next phase (deep
            # pipeline while projections still occupy the PE). chunk-1
            # pairs: PV runs inline, two slots behind the exp, freeing
            # filler room for norms and the out-projection.
            pair_list = [(c, p) for c in range(nqc) for p in range(npr)]
            carry = None        # c0: (c, p, pt), PV pending
            inline_prev = None  # c1: (c, p, box), PV done, norm pending
            tail_fts = {}
            for pi, (c, p) in enumerate(pair_list):
                last = pi == len(pair_list) - 1
                if pi == 0:
                    # rest of kt s0 (needed by this phase's own S slots,
                    # FIFO-ordered first) then next-pair projections and V
                    queue.extend(proj_units("kt", 0, 0, 128, 512))
                    for ch in range(1, nk // 512):
                        queue.extend(proj_units("kt", 0, ch))
                    queue.extend(proj_units("qt", 0, 1))
                    for ch in range(nk // 512):
                        queue.extend(proj_units("kt", 1, ch))
                    for ch in range(nqc):
                        queue.extend(proj_units("qt", 1, ch))
                    for j in range(nj):
                        queue.extend(proj_units("v", j, 0))
                elif pi < npr - 1:
                    for ch in range(nk // 512):
                        queue.extend(proj_units("kt", pi + 1, ch))
                    for ch in range(nqc):
                        queue.extend(proj_units("qt", pi + 1, ch))
                if carry is not None:
                    pc, pp, ppt = carry
                    box, step = make_pv(pc, pp, ppt)
                    for j in range(nj):
                        queue.append((1.0, lambda j=j, s=step: s(j, 0)))
                        queue.append((1.0, lambda j=j, s=step: s(j, 1)))
                    sbc, cgo = make_pvc(box)
                    queue.append((0.0, cgo))
                    queue.append((0.0, make_norm(pc, pp, sbc)))
                    carry = None
                if inline_prev is not None:
                    pc, pp, pbox = inline_prev
                    sbc, cgo = make_pvc(pbox)
                    queue.append((0.0, cgo))
                    queue.append((0.0, make_norm(pc, pp, sbc)))
                    inline_prev = None
                # out-proj of chunk 0 once all its pairs are normalized;
                # first two chunk-1 tiles (sans final accumulation) late
                if nqc == 2:
                    if pi == npr + 1:
                        queue.extend(fout_units(0))
                        queue.extend(fout_units(1))
                    elif pi == npr + 2:
                        queue.extend(fout_units(2))
                        queue.extend(fout_units(3))
                    elif last:
                        for t in (4, 5):
                            tail_fts[t] = fout_units(t, tail=True)
                            queue.extend(tail_fts[t][:npr])

                if pi >= npr and nqc == 2:
                    pt = pt_pool.tile([128, nj, 1024], CDT, tag="pt")
                    box, step = make_pv(c, p, pt)
                    emit_pair(c, p, pt=pt, inline_step=step)
                    step(nj - 2, 0)
                    step(nj - 2, 1)
                    step(nj - 1, 0)
                    step(nj - 1, 1)
                    inline_prev = (c, p, box)
                else:
                    pt = emit_pair(c, p)
                    carry = (c, p, pt)

            # ---- tail: the final pair's 1/l broadcast goes through the
            # PE (ones outer-product) instead of the DRAM bounce; remaining
            # out-proj tiles interleave with the norm chain, and only each
            # tile's final accumulation matmul waits on the last norm.
            drain_all()
            c, p, pbox = inline_prev
            sbc, cgo = make_pvc(pbox)
            cgo()
            rr0 = lr_pool.tile([1, 512], HDT, tag="rr0")
            rr1 = lr_pool.tile([1, 512], HDT, tag="rr1")
            with nc.allow_low_precision(reason="1/l fits fp16 comfortably"):
                nc.vector.reciprocal(out=rr0[0:1, :],
                                     in_=sbc[0][DH:DH + 1, :])
                nc.vector.reciprocal(out=rr1[0:1, :],
                                     in_=sbc[1][DH:DH + 1, :])
            rb_ps = acc_ps.tile([128, 512], FDT, tag="acc", name="rbps")
            nc.tensor.matmul(rb_ps[0:DH, :], lhsT=ones16[0:1, 0:DH],
                             rhs=rr0[0:1, :], start=True, stop=True)
            nc.tensor.matmul(rb_ps[DH:128, :], lhsT=ones16[0:1, 0:DH],
                             rhs=rr1[0:1, :], start=True, stop=True)
            cc = slice(c * 512, (c + 1) * 512)
            nc.vector.tensor_mul(aot_sb[0:DH, p, cc], sbc[0][0:DH, :],
                                 rb_ps[0:DH, :])
            nc.vector.tensor_mul(aot_sb[DH:128, p, cc], sbc[1][0:DH, :],
                                 rb_ps[DH:128, :])
            if nqc == 2:
                for t in (6, 7):
                    tail_fts[t] = fout_units(t, tail=True)
                for _, go in tail_fts[6][:npr]:
                    go()
                tail_fts[4][npr][1]()
                tail_fts[5][npr][1]()
                for _, go in tail_fts[7][:npr]:
                    go()
                tail_fts[6][npr][1]()
                tail_fts[7][npr][1]()
            else:
                for t in range(nrt):
                    for _, go in fout_units(t, tail=True):
                        go()

            rep_stack.close()

    if compile_module:
        nc.compile()
    return nc


def host_inputs(x, x_prev, w_q, w_kv, w_out, b_out, ncores=NCORES):
    """Shard + lay out the full inputs into per-core input maps."""
    bf16 = ml_dtypes.bfloat16
    b, n, dim = x.shape
    inner = w_q.shape[1]
    h = inner // DH
    npr = h // 2
    nq = (b * n) // ncores
    halves = ncores // b
    wq = np.ascontiguousarray(w_q).astype(bf16)
    wkv = np.ascontiguousarray(w_kv).astype(bf16)
    wo = w_out.reshape(npr, 2 * DH, dim)
    wout = np.ascontiguousarray(wo.transpose(1, 0, 2)).astype(bf16)
    bout = np.ascontiguousarray(b_out).reshape(1, dim).astype(bf16)
    in_maps = []
    for c in range(ncores):
        bb, half = c // halves, c % halves
        xt = np.ascontiguousarray(
            x[bb, half * nq:(half + 1) * nq, :].T).astype(bf16)
        xpt = np.ascontiguousarray(x_prev[bb].T).astype(bf16)
        in_maps.append(dict(xt=xt, xpt=xpt, wq=wq, wkv=wkv, wout=wout,
                            bout=bout))
    return in_maps


def _get_module():
    global _BUILT
    if _BUILT is None:
        _BUILT = build_module()
    return _BUILT


def kernel(x, x_prev, w_q, w_kv, w_out, b_out):
    from concourse.bass_utils import run_bass_kernel_spmd

    nc = _get_module()
    in_maps = host_inputs(x, x_prev, w_q, w_kv, w_out, b_out)
    res = run_bass_kernel_spmd(nc, in_maps, core_ids=list(range(NCORES)))

    nq = N // 2
    out = np.empty((B, N, DIM), np.float32)
    for c in range(NCORES):
        b, half = c // 2, c % 2
        out[b, half * nq:(half + 1) * nq, :] = res.results[c]["out"]
    return out
